# revision 25
# baseline (speedup 1.0000x reference)
"""Trainium2 Bass kernel for batched bilinear (general) attention.

Reference computation (all fp32):
    psi = einsum("bth,ah->bta", h_enc, W_psi) + b_psi        # [B, T, A]
    phi = einsum("qbh,ah->qba", h_dec, W_phi) + b_phi        # [Q, B, A]
    e   = einsum("bta,qba->btq", psi, phi)                   # [B, T, Q]
    a   = softmax(e, axis=1)                                 # over T
    c   = einsum("bth,btq->bqh", h_enc, a)                   # [B, Q, H]

Key algebraic refactor: e[b,t,q] = enc_t . M . dec_q + enc_t . u + (per-q const)
with M = W_psi^T @ W_phi [H,H], u = W_psi^T @ b_phi.  Per-q-column constants
are invariant under the softmax over t, so they are dropped.  The host folds
the weights into Z[b] = M @ dec_b^T + u [H, Q] (tiny), and the device only
computes e = enc @ Z, the softmax, and c = enc^T @ softmax(e).

e-phase operand modes:
  f32   - plain fp32 operands (4 cyc/row on PE)
  f32r  - float32r (1 cyc/row, but HW rounds operands to ~13 mantissa bits)
  f16x2 - fp16 hi/lo 3-pass (fp32-accurate, 1 cyc/row)
  f16f8 - split-precision 3-pass: z16*enc16 (main) + z8*enclo8 + zlo16*enc16
          (correction, scaled 2^-11).  fp32-accurate with only 6 bytes/elem
          of enc traffic (fp16 + fp8) instead of 8 (fp32 + fp16).

Sharding: data-parallel over batch B=16 across 8 cores (2 batches per core),
no collectives.
"""

import functools
import os
import sys

import numpy as np

for _p in ("/opt/trn_rl_repo", "/root/.axon_site/_ro/trn_rl_repo"):
    if os.path.isdir(_p) and _p not in sys.path:
        sys.path.append(_p)

B, T, Q, H = 16, 2048, 64, 1024
NCORES = 8
BL = B // NCORES  # batches per core
KT = H // 128  # 8 H-tiles (contraction tiles for e)
NT = T // 128  # 16 T-tiles
NC_CHUNK = T // 512  # 4 chunks of 512 along T for e PSUM banks
SC = 2048.0  # 2^11 scale for the f16f8 correction pass

E_MODE = os.environ.get("ATTN_E_MODE", "f16f8p")
DMA_SPREAD = int(os.environ.get("ATTN_DMA_SPREAD", "2"))


@functools.lru_cache(maxsize=4)
def _build(loop_n: int = 1, e_mode: str = E_MODE, dma_spread: int = DMA_SPREAD):
    import contextlib

    import concourse.mybir as mybir
    import concourse.tile as tile
    from concourse import bacc
    from concourse.bass import ts
    from concourse.masks import make_identity

    f32 = mybir.dt.float32
    f16 = mybir.dt.float16
    f8 = mybir.dt.float8e4
    split = e_mode in ("f16f8", "f16f8p")
    packed = e_mode == "f16f8p"
    e_dt = {"f32": f32, "f32r": mybir.dt.float32r, "f16x2": f16, "f16f8": f16, "f16f8p": f16}[e_mode]
    n_pass = 2 if e_mode == "f16x2" else 1

    nc = bacc.Bacc(
        "TRN2",
        target_bir_lowering=False,
        debug=False,
        enable_asserts=False,
        num_devices=NCORES,
    )

    zw = 2 * Q if packed else Q
    encT_d = nc.dram_tensor("encT", [BL, n_pass, H, T], e_dt, kind="ExternalInput")
    encN_d = nc.dram_tensor("encN", [BL, T, H], f16, kind="ExternalInput")
    z_d = nc.dram_tensor("z", [BL, n_pass, H, zw], e_dt, kind="ExternalInput")
    if split:
        encT8_d = nc.dram_tensor("encT8", [BL, H, T], f8, kind="ExternalInput")
        z8_d = nc.dram_tensor("z8", [BL, H, Q], f8, kind="ExternalInput")
        if not packed:
            zl_d = nc.dram_tensor("zl", [BL, H, Q], f16, kind="ExternalInput")
    c_d = nc.dram_tensor("c", [BL, Q, H], f32, kind="ExternalOutput")

    with tile.TileContext(nc) as tc:
        dma_engines = [nc.sync, nc.scalar, nc.gpsimd, nc.vector][: max(1, dma_spread)]
        _dma_i = [0]

        def dma(out, in_, ring=None):
            if ring is None:
                ring = _dma_i[0] % len(dma_engines)
                _dma_i[0] += 1
            eng = dma_engines[ring % len(dma_engines)]
            eng.dma_start(out=out, in_=in_)

        with (
            tc.tile_pool(name="encT", bufs=3) as p_encT,
            tc.tile_pool(name="encT8", bufs=3) as p_encT8,
            tc.tile_pool(name="encN", bufs=16) as p_encN,
            tc.tile_pool(name="z", bufs=2) as p_z,
            tc.tile_pool(name="eT", bufs=1) as p_eT,
            tc.tile_pool(name="pT", bufs=1) as p_pT,
            tc.tile_pool(name="pN", bufs=2) as p_pN,
            tc.tile_pool(name="outs", bufs=2) as p_out,
            tc.tile_pool(name="stats", bufs=8) as p_stats,
            tc.tile_pool(name="singles", bufs=1) as p_singles,
            tc.tile_pool(name="ps", bufs=8, space="PSUM") as ps,
        ):
            ident = p_singles.tile([64, 64], f32)
            make_identity(nc, ident)

            loop_ctx = (
                tc.For_i(0, loop_n, 1) if loop_n > 1 else contextlib.nullcontext()
            )
            with loop_ctx:
                # ---- phase E: e^T[b] = Z[b]^T @ encT[b]  (contract H) ----
                # k-outer so the T-chunk PSUM banks accumulate while the next
                # encT k-tile streams in (DMA/PE overlap).
                passes = [(0, 0)] if n_pass == 1 else [(0, 0), (0, 1), (1, 0)]
                rs = []
                for b in range(BL):
                    z_t = p_z.tile([128, n_pass, KT, zw], e_dt, tag="z")
                    dma(z_t[:], z_d.ap()[b].rearrange("s (k p) q -> p s k q", p=128))
                    if split:
                        z8_t = p_z.tile([128, KT, Q], f8, tag="z8")
                        dma(z8_t[:], z8_d.ap()[b].rearrange("(k p) q -> p k q", p=128))
                        if not packed:
                            zl_t = p_z.tile([128, KT, Q], f16, tag="zl")
                            dma(zl_t[:], zl_d.ap()[b].rearrange("(k p) q -> p k q", p=128))

                    eT = p_eT.tile([64, T], f32, tag=f"eT{b}")
                    e_pss = [
                        ps.tile([128 if packed else 64, 512], f32, tag="ps", name=f"e_ps_{b}_{c_i}")
                        for c_i in range(NC_CHUNK)
                    ]
                    if split and not packed:
                        e_css = [
                            ps.tile([64, 512], f32, tag="ps", name=f"e_cs_{b}_{c_i}")
                            for c_i in range(NC_CHUNK)
                        ]
                    if packed:
                        GE = 2  # k-tiles per DMA (1 MB fp16 transfers)
                        for kk in range(KT // GE):
                            encT_g = p_encT.tile([128, GE, T], e_dt, tag="encT")
                            dma(
                                encT_g[:],
                                encT_d.ap()[b, 0]
                                .rearrange("(kk g p) t -> kk p g t", g=GE, p=128)[kk],
                            )
                            encT8_g = p_encT8.tile([128, GE, T], f8, tag="encT8")
                            dma(
                                encT8_g[:],
                                encT8_d.ap()[b]
                                .rearrange("(kk g p) t -> kk p g t", g=GE, p=128)[kk],
                            )
                            for g in range(GE):
                                k = kk * GE + g
                                for c_i in range(NC_CHUNK):
                                    nc.tensor.matmul(
                                        e_pss[c_i][:],
                                        lhsT=z_t[:, 0, k, :],
                                        rhs=encT_g[:, g, ts(c_i, 512)],
                                        start=(k == 0),
                                        stop=(k == KT - 1),
                                        skip_group_check=True,
                                    )
                                    nc.tensor.matmul(
                                        e_pss[c_i][64:128, :],
                                        lhsT=z8_t[:, k, :],
                                        rhs=encT8_g[:, g, ts(c_i, 512)],
                                        start=False,
                                        stop=False,
                                        tile_position=(0, 64),
                                        skip_group_check=True,
                                    )
                    for ip, (s0, s1) in enumerate(passes if not packed else []):
                        for k in range(KT):
                            encT_k = p_encT.tile([128, T], e_dt, tag="encT")
                            dma(
                                encT_k[:],
                                encT_d.ap()[b, s1, k * 128 : (k + 1) * 128, :],
                            )
                            if split:
                                encT8_k = p_encT8.tile([128, T], f8, tag="encT8")
                                dma(
                                    encT8_k[:],
                                    encT8_d.ap()[b, k * 128 : (k + 1) * 128, :],
                                )
                            for c_i in range(NC_CHUNK):
                                nc.tensor.matmul(
                                    e_pss[c_i][:],
                                    lhsT=z_t[:, s0, k, :],
                                    rhs=encT_k[:, ts(c_i, 512)],
                                    start=(ip == 0 and k == 0),
                                    stop=(ip == len(passes) - 1 and k == KT - 1),
                                    skip_group_check=split,
                                )
                                if split:
                                    nc.tensor.matmul(
                                        e_css[c_i][:],
                                        lhsT=z8_t[:, k, :],
                                        rhs=encT8_k[:, ts(c_i, 512)],
                                        start=(k == 0),
                                        stop=False,
                                        skip_group_check=True,
                                    )
                                    nc.tensor.matmul(
                                        e_css[c_i][:],
                                        lhsT=zl_t[:, k, :],
                                        rhs=encT_k[:, ts(c_i, 512)],
                                        start=False,
                                        stop=(k == KT - 1),
                                        skip_group_check=True,
                                    )
                    for c_i in range(NC_CHUNK):
                        if not split:
                            if c_i % 2 == 0:
                                nc.vector.tensor_copy(
                                    out=eT[:, ts(c_i, 512)], in_=e_pss[c_i][:]
                                )
                            else:
                                nc.scalar.copy(out=eT[:, ts(c_i, 512)], in_=e_pss[c_i][:])
                        else:
                            # eT = main + corr / SC
                            corr_ap = (
                                e_pss[c_i][64:128, :] if packed else e_css[c_i][:]
                            )
                            main_ap = (
                                e_pss[c_i][0:64, :] if packed else e_pss[c_i][:]
                            )
                            nc.scalar.activation(
                                out=eT[:, ts(c_i, 512)],
                                in_=corr_ap,
                                func=mybir.ActivationFunctionType.Copy,
                                bias=0.0,
                                scale=1.0 / SC,
                            )
                            nc.vector.tensor_add(
                                eT[:, ts(c_i, 512)], eT[:, ts(c_i, 512)], main_ap
                            )

                    # ---- phase S: softmax stats over T (free dim) ----
                    negm = p_stats.tile([64, 1], f32, tag="negm")
                    nc.vector.reduce_max(
                        out=negm[:], in_=eT[:], axis=mybir.AxisListType.X, negate=True
                    )
                    pT = p_pT.tile([64, T], f32, tag=f"pT{b}")
                    s_sum = p_stats.tile([64, 1], f32, tag="s")
                    nc.scalar.activation(
                        out=pT[:],
                        in_=eT[:],
                        func=mybir.ActivationFunctionType.Exp,
                        bias=negm[:],
                        scale=1.0,
                        accum_out=s_sum[:],
                    )
                    r = p_stats.tile([64, 1], f32, tag="r")
                    nc.vector.reciprocal(out=r[:], in_=s_sum[:])
                    rs.append((pT, r))

                # ---- phases T + C per batch ----
                for b in range(BL):
                    pT, r = rs[b]
                    # transpose p^T [64, T] -> p natural tiles [128, 64] (fp16)
                    pN = p_pN.tile([128, NT, Q], f16, tag="pN")
                    for tt in range(NT):
                        tr_ps = ps.tile([128, 64], f32, tag="ps", name=f"tr_ps_{b}_{tt}")
                        nc.tensor.transpose(
                            out=tr_ps[:], in_=pT[:, ts(tt, 128)], identity=ident[:]
                        )
                        nc.vector.tensor_copy(out=pN[:, tt, :], in_=tr_ps[:])

                    # c[b] = p^T @ encN[b]  (contract T), scaled by r on evac
                    c_ps0 = ps.tile([64, 512], f32, tag="ps", name=f"c_ps0_{b}")
                    c_ps1 = ps.tile([64, 512], f32, tag="ps", name=f"c_ps1_{b}")
                    GC = 2  # t-tiles per DMA (512 KB transfers)
                    for tg in range(NT // GC):
                        encN_t = p_encN.tile([128, GC, H], f16, tag="encN")
                        dma(
                            encN_t[:],
                            encN_d.ap()[b]
                            .rearrange("(tg g p) h -> tg p g h", g=GC, p=128)[tg],
                        )
                        for g in range(GC):
                            tt = tg * GC + g
                            nc.tensor.matmul(
                                c_ps0[:],
                                lhsT=pN[:, tt, :],
                                rhs=encN_t[:, g, 0:512],
                                start=(tt == 0),
                                stop=(tt == NT - 1),
                            )
                            nc.tensor.matmul(
                                c_ps1[:],
                                lhsT=pN[:, tt, :],
                                rhs=encN_t[:, g, 512:1024],
                                start=(tt == 0),
                                stop=(tt == NT - 1),
                            )

                    out_t = p_out.tile([64, H], f32, tag="out")
                    nc.vector.tensor_scalar_mul(out_t[:, 0:512], c_ps0[:], r[:])
                    nc.vector.tensor_scalar_mul(out_t[:, 512:1024], c_ps1[:], r[:])
                    dma(c_d.ap()[b], out_t[:])

    nc.compile()
    return nc


def _split_f16(x):
    hi = x.astype(np.float16)
    lo = (x - hi.astype(np.float32)).astype(np.float16)
    return hi, lo


def _host_prep(h_enc, h_dec, W_psi, b_psi, W_phi, b_phi, e_mode: str = E_MODE):
    import ml_dtypes

    f8 = ml_dtypes.float8_e4m3

    h_enc = np.asarray(h_enc, dtype=np.float32)
    h_dec = np.asarray(h_dec, dtype=np.float32)
    W_psi = np.asarray(W_psi, dtype=np.float64)
    W_phi = np.asarray(W_phi, dtype=np.float64)
    b_phi = np.asarray(b_phi, dtype=np.float64)

    # M = W_psi^T @ W_phi [H, H];  u = W_psi^T @ b_phi [H]
    M = W_psi.T @ W_phi
    u = W_psi.T @ b_phi
    # Z[b, h, q] = sum_k M[h, k] * h_dec[q, b, k] + u[h]
    dec_r = h_dec.astype(np.float64).transpose(2, 1, 0).reshape(H, B * Q)
    Z = (M @ dec_r).reshape(H, B, Q).transpose(1, 0, 2) + u[None, :, None]
    Z = np.ascontiguousarray(Z, dtype=np.float32)  # [B, H, Q]

    encT = np.ascontiguousarray(h_enc.transpose(0, 2, 1))  # [B, H, T] fp32
    arrays = {"encN": h_enc.astype(np.float16)}
    if e_mode == "f16x2":
        arrays["encT"] = np.stack(_split_f16(encT), axis=1)
        arrays["z"] = np.stack(_split_f16(Z), axis=1)
    elif e_mode in ("f16f8", "f16f8p"):
        eh = encT.astype(np.float16)
        arrays["encT"] = eh[:, None]
        arrays["encT8"] = ((encT - eh.astype(np.float32)) * SC).astype(f8)
        zh = Z.astype(np.float16)
        zl = ((Z - zh.astype(np.float32)) * SC).astype(np.float16)
        arrays["z8"] = Z.astype(f8)
        if e_mode == "f16f8p":
            arrays["z"] = np.concatenate([zh, zl], axis=2)[:, None]  # [B,1,H,2Q]
        else:
            arrays["z"] = zh[:, None]
            arrays["zl"] = zl
    else:
        arrays["encT"] = encT[:, None]
        arrays["z"] = Z[:, None]
    return arrays


def _in_maps(arrays):
    maps = []
    for i in range(NCORES):
        s = slice(i * BL, (i + 1) * BL)
        maps.append({k: v[s] for k, v in arrays.items()})
    return maps


def kernel(h_enc, h_dec, W_psi, b_psi, W_phi, b_phi):
    from concourse.bass_utils import run_bass_kernel_spmd

    arrays = _host_prep(h_enc, h_dec, W_psi, b_psi, W_phi, b_phi)
    nc = _build()
    res = run_bass_kernel_spmd(nc, _in_maps(arrays), core_ids=list(range(NCORES)))
    out = np.concatenate([res.results[i]["c"] for i in range(NCORES)], axis=0)
    return np.ascontiguousarray(out, dtype=np.float32)


# revision 26
# speedup vs baseline: 1.1932x; 1.1932x over previous
"""Trainium2 Bass kernel for batched bilinear (general) attention.

Reference computation (all fp32):
    psi = einsum("bth,ah->bta", h_enc, W_psi) + b_psi        # [B, T, A]
    phi = einsum("qbh,ah->qba", h_dec, W_phi) + b_phi        # [Q, B, A]
    e   = einsum("bta,qba->btq", psi, phi)                   # [B, T, Q]
    a   = softmax(e, axis=1)                                 # over T
    c   = einsum("bth,btq->bqh", h_enc, a)                   # [B, Q, H]

Key algebraic refactor: e[b,t,q] = enc_t . M . dec_q + enc_t . u + (per-q const)
with M = W_psi^T @ W_phi [H,H], u = W_psi^T @ b_phi.  Per-q-column constants
are invariant under the softmax over t, so they are dropped.  The host folds
the weights into Z[b] = M @ dec_b^T + u [H, Q] (tiny), and the device only
computes e = enc @ Z, the softmax, and c = enc^T @ softmax(e).

e-phase operand modes:
  f32   - plain fp32 operands (4 cyc/row on PE)
  f32r  - float32r (1 cyc/row, but HW rounds operands to ~13 mantissa bits)
  f16x2 - fp16 hi/lo 3-pass (fp32-accurate, 1 cyc/row)
  f16f8 - split-precision 3-pass: z16*enc16 (main) + z8*enclo8 + zlo16*enc16
          (correction, scaled 2^-11).  fp32-accurate with only 6 bytes/elem
          of enc traffic (fp16 + fp8) instead of 8 (fp32 + fp16).

Sharding: data-parallel over batch B=16 across 8 cores (2 batches per core),
no collectives.
"""

import functools
import os
import sys

import numpy as np

for _p in ("/opt/trn_rl_repo", "/root/.axon_site/_ro/trn_rl_repo"):
    if os.path.isdir(_p) and _p not in sys.path:
        sys.path.append(_p)

B, T, Q, H = 16, 2048, 64, 1024
NCORES = 8
BL = B // NCORES  # batches per core
KT = H // 128  # 8 H-tiles (contraction tiles for e)
NT = T // 128  # 16 T-tiles
NC_CHUNK = T // 512  # 4 chunks of 512 along T for e PSUM banks
SC = 2048.0  # 2^11 scale for the f16f8 correction pass
GE = 2  # k-tiles per encT DMA (1 MB fp16 transfers, 8 KB contiguous/partition)
GC = 2  # t-tiles per encN DMA (512 KB transfers, 4 KB contiguous/partition)

E_MODE = os.environ.get("ATTN_E_MODE", "f16f8p")
DMA_SPREAD = int(os.environ.get("ATTN_DMA_SPREAD", "2"))


@functools.lru_cache(maxsize=4)
def _build(loop_n: int = 1, e_mode: str = E_MODE, dma_spread: int = DMA_SPREAD):
    import contextlib

    import concourse.mybir as mybir
    import concourse.tile as tile
    from concourse import bacc
    from concourse.bass import ts
    from concourse.masks import make_identity

    f32 = mybir.dt.float32
    f16 = mybir.dt.float16
    f8 = mybir.dt.float8e4
    split = e_mode in ("f16f8", "f16f8p")
    packed = e_mode == "f16f8p"
    e_dt = {"f32": f32, "f32r": mybir.dt.float32r, "f16x2": f16, "f16f8": f16, "f16f8p": f16}[e_mode]
    n_pass = 2 if e_mode == "f16x2" else 1

    nc = bacc.Bacc(
        "TRN2",
        target_bir_lowering=False,
        debug=False,
        enable_asserts=False,
        num_devices=NCORES,
    )

    zw = 2 * Q if packed else Q
    if packed:
        # host-interleaved tiled layouts: each partition's slice of a DMA is
        # one contiguous run (GE*T*2 = 8 KB fp16), maximizing DMA efficiency
        encT_d = nc.dram_tensor(
            "encT", [BL, KT // GE, 128, GE * T], f16, kind="ExternalInput"
        )
        encN_d = nc.dram_tensor(
            "encN", [BL, NT // GC, 128, GC * H], f16, kind="ExternalInput"
        )
        encT8_d = nc.dram_tensor(
            "encT8", [BL, KT // GE, 128, GE * T], f8, kind="ExternalInput"
        )
    else:
        encT_d = nc.dram_tensor("encT", [BL, n_pass, H, T], e_dt, kind="ExternalInput")
        encN_d = nc.dram_tensor("encN", [BL, T, H], f16, kind="ExternalInput")
        if split:
            encT8_d = nc.dram_tensor("encT8", [BL, H, T], f8, kind="ExternalInput")
    z_d = nc.dram_tensor("z", [BL, n_pass, H, zw], e_dt, kind="ExternalInput")
    if split:
        z8_d = nc.dram_tensor("z8", [BL, H, Q], f8, kind="ExternalInput")
        if not packed:
            zl_d = nc.dram_tensor("zl", [BL, H, Q], f16, kind="ExternalInput")
    c_d = nc.dram_tensor("c", [BL, Q, H], f32, kind="ExternalOutput")

    with tile.TileContext(nc) as tc:
        dma_engines = [nc.sync, nc.scalar, nc.gpsimd, nc.vector][: max(1, dma_spread)]
        _dma_i = [0]

        def dma(out, in_, ring=None):
            if ring is None:
                ring = _dma_i[0] % len(dma_engines)
                _dma_i[0] += 1
            eng = dma_engines[ring % len(dma_engines)]
            eng.dma_start(out=out, in_=in_)

        with (
            tc.tile_pool(name="encT", bufs=3) as p_encT,
            tc.tile_pool(name="encT8", bufs=3) as p_encT8,
            tc.tile_pool(name="encN", bufs=16) as p_encN,
            tc.tile_pool(name="z", bufs=2) as p_z,
            tc.tile_pool(name="eT", bufs=1) as p_eT,
            tc.tile_pool(name="pT", bufs=1) as p_pT,
            tc.tile_pool(name="pN", bufs=2) as p_pN,
            tc.tile_pool(name="outs", bufs=2) as p_out,
            tc.tile_pool(name="stats", bufs=8) as p_stats,
            tc.tile_pool(name="singles", bufs=1) as p_singles,
            tc.tile_pool(name="ps", bufs=8, space="PSUM") as ps,
        ):
            ident = p_singles.tile([64, 64], f32)
            make_identity(nc, ident)

            loop_ctx = (
                tc.For_i(0, loop_n, 1) if loop_n > 1 else contextlib.nullcontext()
            )
            with loop_ctx:
                # ---- phase E: e^T[b] = Z[b]^T @ encT[b]  (contract H) ----
                # k-outer so the T-chunk PSUM banks accumulate while the next
                # encT k-tile streams in (DMA/PE overlap).
                passes = [(0, 0)] if n_pass == 1 else [(0, 0), (0, 1), (1, 0)]
                rs = []
                for b in range(BL):
                    z_t = p_z.tile([128, n_pass, KT, zw], e_dt, tag="z")
                    dma(z_t[:], z_d.ap()[b].rearrange("s (k p) q -> p s k q", p=128))
                    if split:
                        z8_t = p_z.tile([128, KT, Q], f8, tag="z8")
                        dma(z8_t[:], z8_d.ap()[b].rearrange("(k p) q -> p k q", p=128))
                        if not packed:
                            zl_t = p_z.tile([128, KT, Q], f16, tag="zl")
                            dma(zl_t[:], zl_d.ap()[b].rearrange("(k p) q -> p k q", p=128))

                    eT = p_eT.tile([64, T], f32, tag=f"eT{b}")
                    e_pss = [
                        ps.tile([128 if packed else 64, 512], f32, tag="ps", name=f"e_ps_{b}_{c_i}")
                        for c_i in range(NC_CHUNK)
                    ]
                    if split and not packed:
                        e_css = [
                            ps.tile([64, 512], f32, tag="ps", name=f"e_cs_{b}_{c_i}")
                            for c_i in range(NC_CHUNK)
                        ]
                    if packed:
                        for kk in range(KT // GE):
                            encT_g = p_encT.tile([128, GE * T], f16, tag="encT")
                            dma(encT_g[:], encT_d.ap()[b, kk])
                            encT8_g = p_encT8.tile([128, GE * T], f8, tag="encT8")
                            dma(encT8_g[:], encT8_d.ap()[b, kk])
                            for g in range(GE):
                                k = kk * GE + g
                                for c_i in range(NC_CHUNK):
                                    nc.tensor.matmul(
                                        e_pss[c_i][:],
                                        lhsT=z_t[:, 0, k, :],
                                        rhs=encT_g[:, ts(g * NC_CHUNK + c_i, 512)],
                                        start=(k == 0),
                                        stop=(k == KT - 1),
                                        skip_group_check=True,
                                    )
                                    nc.tensor.matmul(
                                        e_pss[c_i][64:128, :],
                                        lhsT=z8_t[:, k, :],
                                        rhs=encT8_g[:, ts(g * NC_CHUNK + c_i, 512)],
                                        start=False,
                                        stop=False,
                                        tile_position=(0, 64),
                                        skip_group_check=True,
                                    )
                    for ip, (s0, s1) in enumerate(passes if not packed else []):
                        for k in range(KT):
                            encT_k = p_encT.tile([128, T], e_dt, tag="encT")
                            dma(
                                encT_k[:],
                                encT_d.ap()[b, s1, k * 128 : (k + 1) * 128, :],
                            )
                            if split:
                                encT8_k = p_encT8.tile([128, T], f8, tag="encT8")
                                dma(
                                    encT8_k[:],
                                    encT8_d.ap()[b, k * 128 : (k + 1) * 128, :],
                                )
                            for c_i in range(NC_CHUNK):
                                nc.tensor.matmul(
                                    e_pss[c_i][:],
                                    lhsT=z_t[:, s0, k, :],
                                    rhs=encT_k[:, ts(c_i, 512)],
                                    start=(ip == 0 and k == 0),
                                    stop=(ip == len(passes) - 1 and k == KT - 1),
                                    skip_group_check=split,
                                )
                                if split:
                                    nc.tensor.matmul(
                                        e_css[c_i][:],
                                        lhsT=z8_t[:, k, :],
                                        rhs=encT8_k[:, ts(c_i, 512)],
                                        start=(k == 0),
                                        stop=False,
                                        skip_group_check=True,
                                    )
                                    nc.tensor.matmul(
                                        e_css[c_i][:],
                                        lhsT=zl_t[:, k, :],
                                        rhs=encT_k[:, ts(c_i, 512)],
                                        start=False,
                                        stop=(k == KT - 1),
                                        skip_group_check=True,
                                    )
                    for c_i in range(NC_CHUNK):
                        if not split:
                            if c_i % 2 == 0:
                                nc.vector.tensor_copy(
                                    out=eT[:, ts(c_i, 512)], in_=e_pss[c_i][:]
                                )
                            else:
                                nc.scalar.copy(out=eT[:, ts(c_i, 512)], in_=e_pss[c_i][:])
                        else:
                            # eT = main + corr / SC
                            corr_ap = (
                                e_pss[c_i][64:128, :] if packed else e_css[c_i][:]
                            )
                            main_ap = (
                                e_pss[c_i][0:64, :] if packed else e_pss[c_i][:]
                            )
                            nc.scalar.activation(
                                out=eT[:, ts(c_i, 512)],
                                in_=corr_ap,
                                func=mybir.ActivationFunctionType.Copy,
                                bias=0.0,
                                scale=1.0 / SC,
                            )
                            nc.vector.tensor_add(
                                eT[:, ts(c_i, 512)], eT[:, ts(c_i, 512)], main_ap
                            )

                    # ---- phase S: softmax stats over T (free dim) ----
                    negm = p_stats.tile([64, 1], f32, tag="negm")
                    nc.vector.reduce_max(
                        out=negm[:], in_=eT[:], axis=mybir.AxisListType.X, negate=True
                    )
                    pT = p_pT.tile([64, T], f32, tag=f"pT{b}")
                    s_sum = p_stats.tile([64, 1], f32, tag="s")
                    nc.scalar.activation(
                        out=pT[:],
                        in_=eT[:],
                        func=mybir.ActivationFunctionType.Exp,
                        bias=negm[:],
                        scale=1.0,
                        accum_out=s_sum[:],
                    )
                    r = p_stats.tile([64, 1], f32, tag="r")
                    nc.vector.reciprocal(out=r[:], in_=s_sum[:])
                    rs.append((pT, r))

                # ---- phases T + C per batch ----
                for b in range(BL):
                    pT, r = rs[b]
                    # transpose p^T [64, T] -> p natural tiles [128, 64] (fp16)
                    pN = p_pN.tile([128, NT, Q], f16, tag="pN")
                    for tt in range(NT):
                        tr_ps = ps.tile([128, 64], f32, tag="ps", name=f"tr_ps_{b}_{tt}")
                        nc.tensor.transpose(
                            out=tr_ps[:], in_=pT[:, ts(tt, 128)], identity=ident[:]
                        )
                        nc.vector.tensor_copy(out=pN[:, tt, :], in_=tr_ps[:])

                    # c[b] = p^T @ encN[b]  (contract T), scaled by r on evac
                    c_ps0 = ps.tile([64, 512], f32, tag="ps", name=f"c_ps0_{b}")
                    c_ps1 = ps.tile([64, 512], f32, tag="ps", name=f"c_ps1_{b}")
                    for tg in range(NT // GC):
                        encN_t = p_encN.tile([128, GC * H], f16, tag="encN")
                        dma(encN_t[:], encN_d.ap()[b, tg])
                        for g in range(GC):
                            tt = tg * GC + g
                            nc.tensor.matmul(
                                c_ps0[:],
                                lhsT=pN[:, tt, :],
                                rhs=encN_t[:, ts(2 * g, 512)],
                                start=(tt == 0),
                                stop=(tt == NT - 1),
                            )
                            nc.tensor.matmul(
                                c_ps1[:],
                                lhsT=pN[:, tt, :],
                                rhs=encN_t[:, ts(2 * g + 1, 512)],
                                start=(tt == 0),
                                stop=(tt == NT - 1),
                            )

                    out_t = p_out.tile([64, H], f32, tag="out")
                    nc.vector.tensor_scalar_mul(out_t[:, 0:512], c_ps0[:], r[:])
                    nc.vector.tensor_scalar_mul(out_t[:, 512:1024], c_ps1[:], r[:])
                    dma(c_d.ap()[b], out_t[:])

    nc.compile()
    return nc


def _split_f16(x):
    hi = x.astype(np.float16)
    lo = (x - hi.astype(np.float32)).astype(np.float16)
    return hi, lo


def _host_prep(h_enc, h_dec, W_psi, b_psi, W_phi, b_phi, e_mode: str = E_MODE):
    import ml_dtypes

    f8 = ml_dtypes.float8_e4m3

    h_enc = np.asarray(h_enc, dtype=np.float32)
    h_dec = np.asarray(h_dec, dtype=np.float32)
    W_psi = np.asarray(W_psi, dtype=np.float64)
    W_phi = np.asarray(W_phi, dtype=np.float64)
    b_phi = np.asarray(b_phi, dtype=np.float64)

    # M = W_psi^T @ W_phi [H, H];  u = W_psi^T @ b_phi [H]
    M = W_psi.T @ W_phi
    u = W_psi.T @ b_phi
    # Z[b, h, q] = sum_k M[h, k] * h_dec[q, b, k] + u[h]
    dec_r = h_dec.astype(np.float64).transpose(2, 1, 0).reshape(H, B * Q)
    Z = (M @ dec_r).reshape(H, B, Q).transpose(1, 0, 2) + u[None, :, None]
    Z = np.ascontiguousarray(Z, dtype=np.float32)  # [B, H, Q]

    encT = np.ascontiguousarray(h_enc.transpose(0, 2, 1))  # [B, H, T] fp32
    arrays = {"encN": h_enc.astype(np.float16)}
    if e_mode == "f16x2":
        arrays["encT"] = np.stack(_split_f16(encT), axis=1)
        arrays["z"] = np.stack(_split_f16(Z), axis=1)
    elif e_mode in ("f16f8", "f16f8p"):
        eh = encT.astype(np.float16)
        el8 = ((encT - eh.astype(np.float32)) * SC).astype(f8)
        zh = Z.astype(np.float16)
        zl = ((Z - zh.astype(np.float32)) * SC).astype(np.float16)
        arrays["z8"] = Z.astype(f8)
        if e_mode == "f16f8p":
            def tile_i(x, g):  # [B, G*g*128, W] -> [B, G, 128, g*W] interleaved
                Bn, R, W = x.shape
                G = R // (g * 128)
                return np.ascontiguousarray(
                    x.reshape(Bn, G, g, 128, W).transpose(0, 1, 3, 2, 4).reshape(
                        Bn, G, 128, g * W
                    )
                )

            arrays["encT"] = tile_i(eh, GE)
            arrays["encT8"] = tile_i(el8, GE)
            arrays["encN"] = tile_i(arrays["encN"], GC)
            arrays["z"] = np.concatenate([zh, zl], axis=2)[:, None]  # [B,1,H,2Q]
        else:
            arrays["encT"] = eh[:, None]
            arrays["encT8"] = el8
            arrays["z"] = zh[:, None]
            arrays["zl"] = zl
    else:
        arrays["encT"] = encT[:, None]
        arrays["z"] = Z[:, None]
    return arrays


def _in_maps(arrays):
    maps = []
    for i in range(NCORES):
        s = slice(i * BL, (i + 1) * BL)
        maps.append({k: v[s] for k, v in arrays.items()})
    return maps


def kernel(h_enc, h_dec, W_psi, b_psi, W_phi, b_phi):
    from concourse.bass_utils import run_bass_kernel_spmd

    arrays = _host_prep(h_enc, h_dec, W_psi, b_psi, W_phi, b_phi)
    nc = _build()
    res = run_bass_kernel_spmd(nc, _in_maps(arrays), core_ids=list(range(NCORES)))
    out = np.concatenate([res.results[i]["c"] for i in range(NCORES)], axis=0)
    return np.ascontiguousarray(out, dtype=np.float32)
